# revision 34
# baseline (speedup 1.0000x reference)
"""Multi-head attention kernel for 8 TRN2 NeuronCores.

Problem: B=2, S=2048, D=1024, H=16 heads, head_dim=64, fp32 I/O.

Sharding: 8 cores = 2 batches x 4 head-groups. Core c handles batch c//4 and
heads [4*(c%4), 4*(c%4)+4). Host sums the 4 bf16 partials per batch and adds
the output bias in fp32.

Per-core structure (v2 — rescheduled from the 236us baseline, now ~222us):
  - exp on ScalarE is the pacer: 128 slots x ~1147ns (ACT floor ~147us).
  - Projection / v / out-proj work drips into the slot stream as quanta
    placed by a build-time EDF greedy with data-arrival gating and a strict
    global ordering lane for the single-buffered qk psum accumulator.
  - The softmax denominator reciprocal is DEFERRED into the next block and
    computed via DVE 32x32 block transposes (SBUF-local; no DMA-bounce
    latency): Z row -> 32 lanes -> 8-cyc/elem reciprocal -> row again, then
    a K=1 PE matmul broadcasts it for the normalize multiply. Pair-0 blocks
    normalize in late slots (their out_proj runs 4+ blocks later), pair-1 in
    early slots.
  - attn@v lags its exp by >=1 slot (never stalls the in-order PE queue on
    the just-issued exp) and defers per-jc until its vt chunk exists, so
    block 0 exp's before any v projection is done. The last attn@v of a
    block plus the psum evacuation are emitted after the next block's first
    scores (psum-group-safe variant).
  - Input DMAs: [wk|wq] merged (one completion gates both), x sc0 split
    across the gpsimd+scalar rings, sc1-3 behind sc0 on gpsimd. Junk
    matmuls on memset data hold the PE HAM at full clock through the
    DMA-paced prologue; k0/q0 first-chunk matmuls interleave so q0's first
    half runs during the sc0 hi-half DMA.
  - Tail: final-ic out_proj uses pre-staged at0 partials (dripped during
    blocks 5-7) plus two K=64 head matmuls and a DVE add; copies alternate
    ScalarE/VectorE; reserved ic2 chunks + junk keep the PE warm through
    the final transpose-recip chain.
"""

import numpy as np
import ml_dtypes

import concourse.mybir as mybir
import concourse.tile as tile
from concourse import bacc
from concourse.bass_utils import run_bass_kernel_spmd

BF16 = mybir.dt.bfloat16
FP32 = mybir.dt.float32

B, S, D = 2, 2048, 1024
NH, DH = 16, 64
NCORES = 8
GROUPS = 4                 # head-groups (cores per batch)
HL = NH // GROUPS          # heads per core = 4
FL = HL * DH               # features per core = 256

SC = 512                   # i/s chunk (PSUM bank = 512 fp32)
JC = 128                   # j chunk (partition dim)
DCH = D // 128             # contraction chunks over embed dim = 8
N_SC = S // SC             # 4
N_JC = S // JC             # 16

# biasblob column offsets
BB_BK, BB_BQ, BB_BV, BB_MB = 0, 2, 4, 4 + FL
BB_COLS = 4 + FL + N_JC


def build_kernel():
    nc = bacc.Bacc("TRN2", target_bir_lowering=False, debug=False)

    # xT packed sc-major: col = sc*(DCH*SC) + dc*SC + s
    xTp = nc.dram_tensor("xTp", [128, N_SC * DCH * SC], BF16, kind="ExternalInput")
    # [wk | wq] — one DMA, one completion for both projection weights
    wkq = nc.dram_tensor("wkq", [128, 2 * DCH * FL], BF16, kind="ExternalInput")
    bb = nc.dram_tensor("bb", [128, BB_COLS], FP32, kind="ExternalInput")
    wv = nc.dram_tensor("wv", [128, DCH * FL], BF16, kind="ExternalInput")
    # wo4 = [fc0 | fc1 | fc1 headC @0:64 | fc1 headD @0:64]
    wo = nc.dram_tensor("wo", [128, 4 * D], BF16, kind="ExternalInput")
    out = nc.dram_tensor("out", [S, D], BF16, kind="ExternalOutput")

    with tile.TileContext(nc) as tc:
        with (
            tc.tile_pool(name="weights", bufs=1) as wpool,
            tc.tile_pool(name="acts", bufs=1) as apool,
            tc.tile_pool(name="exps", bufs=16) as epool,
            tc.tile_pool(name="stages", bufs=6) as spool,
            tc.tile_pool(name="osbs", bufs=6) as opool,
            tc.tile_pool(name="smalls", bufs=6) as smpool,
            tc.tile_pool(name="scores", bufs=2, space="PSUM") as scpool,
            tc.tile_pool(name="attnout", bufs=2, space="PSUM") as aopool,
            tc.tile_pool(name="qkacc", bufs=1, space="PSUM") as qkpool,
            tc.tile_pool(name="transient", bufs=1, space="PSUM") as trpool,
        ):
            # ---- resident SBUF ----
            xt_all = wpool.tile([128, N_SC * DCH * SC], BF16, name="xt_all")

            def xt_sl(scn, dc, lo=0, hi=SC):
                base = (scn * DCH + dc) * SC
                return xt_all[:, base + lo:base + hi]

            WKQ = DCH * FL
            wkq_sb = wpool.tile([128, 2 * WKQ], BF16, name="wkq_sb")
            wkt = [wkq_sb[:, dc * FL:(dc + 1) * FL] for dc in range(DCH)]
            wqt = [wkq_sb[:, WKQ + dc * FL:WKQ + (dc + 1) * FL] for dc in range(DCH)]
            bb_sb = wpool.tile([128, BB_COLS], FP32, name="bb_sb")
            bk_sb = bb_sb[:, BB_BK:BB_BK + 2]
            bq_sb = bb_sb[:, BB_BQ:BB_BQ + 2]
            bv_sb = bb_sb[:, BB_BV:BB_BV + FL]
            mb_sb = bb_sb[:, BB_MB:BB_MB + N_JC]
            wv_sb = wpool.tile([128, DCH * FL], BF16, name="wv_sb")
            wvt = [wv_sb[:, dc * FL:(dc + 1) * FL] for dc in range(DCH)]
            wo_sb = wpool.tile([128, 4 * D], BF16, name="wo_sb")

            # ones row at partition 0 for the recip broadcast matmul
            ones0 = wpool.tile([1, 64], BF16, name="ones0")
            nc.vector.memset(ones0, 1.0)
            # junk operand for PE warm-up matmuls (must not depend on DMA)
            warm_sb = wpool.tile([128, SC], BF16, name="warm_sb")
            nc.vector.memset(warm_sb, 0.0)

            # ---- input DMAs, priority order ----
            # Rings are FIFO per issuing engine while the 16 SDMA engines
            # round-robin across rings, so each ring is a priority lane:
            #   sync:   bb, wk, wq, wv, wo   (first exp needs wk/wq/bb)
            #   gpsimd: sc0 lo-half, then sc1, sc2, sc3 (x stream, in order)
            #   scalar: sc0 hi-half only (keeps the ACT queue free for exps)
            nc.sync.dma_start(out=wkq_sb, in_=wkq.ap())
            nc.gpsimd.dma_start(
                out=xt_all[:, 0:4 * SC],
                in_=xTp.ap()[0:128, 0:4 * SC])
            nc.scalar.dma_start(
                out=xt_all[:, 4 * SC:8 * SC],
                in_=xTp.ap()[0:128, 4 * SC:8 * SC])
            nc.sync.dma_start(out=bb_sb, in_=bb.ap())
            nc.sync.dma_start(out=wv_sb, in_=wv.ap())
            nc.sync.dma_start(out=wo_sb, in_=wo.ap())
            # x s-chunks 1-3 on the gpsimd ring behind sc0's lo half — the
            # early flood costs the first exp a few us but every drip's data
            # is resident long before its slot (measured best overall).
            for scn in (1, 2, 3):
                base = scn * DCH * SC
                nc.gpsimd.dma_start(
                    out=xt_all[:, base:base + DCH * SC],
                    in_=xTp.ap()[0:128, base:base + DCH * SC])

            # warm the ScalarE Exp table while DMAs stream
            warm = smpool.tile([1, 4], FP32, name="warm", tag="warm")
            nc.vector.memset(warm, 1.0)
            nc.scalar.activation(warm, warm, mybir.ActivationFunctionType.Exp)

            # PE warm-up: keep HAM busy so real projections run at 2.4GHz
            warm_ps = trpool.tile([128, SC], FP32, name="warm_ps", tag="tr")
            for _ in range(12):
                nc.tensor.matmul(warm_ps, lhsT=warm_sb[:, 0:128], rhs=warm_sb)

            def junk_mms(n):
                # scores psum ring is free whenever junk runs (prologue/tail)
                jp = scpool.tile([128, 2 * SC], FP32, name="sc_ps")[:, 0:SC]
                for _ in range(n):
                    nc.tensor.matmul(jp, lhsT=warm_sb[:, 0:128], rhs=warm_sb)

            # ---- persistent activations ----
            qt = [apool.tile([128, S], BF16, name=f"qt{p}") for p in range(2)]
            kt = [apool.tile([128, S], BF16, name=f"kt{p}") for p in range(2)]
            vt = [apool.tile([128, HL, 65], BF16, name=f"vt{j}") for j in range(N_JC)]
            at = [apool.tile([128, S], BF16, name=f"at{p}") for p in range(2)]
            stg_tail = apool.tile([64, SC], BF16, name="stg_tail")
            # normalize scratch: per-block osb tiles + one transpose set.
            # Rows 64:96 and the rsp tiles are memset once so the 32x32
            # block transposes never read never-written bytes (row 64 is
            # rewritten by each block's psum copy before any transpose).
            osb_ring = [apool.tile([96, SC], FP32, name=f"osb{i}") for i in range(16)]
            for t in osb_ring:
                nc.vector.memset(t[64:96, :], 0.0)
            zt_pair = [apool.tile([32, SC], FP32, name=f"zt{h}") for h in range(2)]
            rsp_pair = [apool.tile([32, SC], BF16, name=f"rsp{h}") for h in range(2)]
            for t in rsp_pair:
                nc.vector.memset(t, 0.0)
            rec_pair = [apool.tile([32, SC], BF16, name=f"rec{h}") for h in range(2)]



            qk_open = [None]

            def qk_quarter(dst, w_tiles, bias_sb, scn, fc, qq):
                """Quarter of a qT/kT projection s-chunk (2 of 8 d-matmuls)."""
                if qq == 0:
                    ps = qkpool.tile([128, SC], FP32, name="qk_ps", tag="qk")
                    qk_open[0] = ps
                else:
                    ps = qk_open[0]
                for dc in range(qq * 2, qq * 2 + 2):
                    nc.tensor.matmul(
                        ps,
                        lhsT=w_tiles[dc][:, fc * 128:(fc + 1) * 128],
                        rhs=xt_sl(scn, dc),
                        start=(dc == 0),
                        stop=(dc == DCH - 1),
                    )
                if qq == 3:
                    nc.vector.tensor_scalar_add(
                        dst[:, scn * SC:(scn + 1) * SC], ps, bias_sb[:, fc:fc + 1]
                    )

            def qk_full(dst, w_tiles, bias_sb, scn, fc):
                for qq in range(4):
                    qk_quarter(dst, w_tiles, bias_sb, scn, fc, qq)

            def v_proj(j, pair):
                """v rows [128j,+128) for one head-pair (N=128)."""
                ps = trpool.tile([128, 128], FP32, name="v_ps", tag="tr")
                scn, off = j // 4, (j % 4) * JC
                for dc in range(DCH):
                    nc.tensor.matmul(
                        ps,
                        lhsT=xt_sl(scn, dc, off, off + JC),
                        rhs=wvt[dc][:, pair * 128:(pair + 1) * 128],
                        start=(dc == 0),
                        stop=(dc == DCH - 1),
                    )
                nc.vector.tensor_add(
                    vt[j][:, 2 * pair:2 * pair + 2, 0:64],
                    ps.rearrange("p (h d) -> p h d", h=2),
                    bv_sb[:, pair * 128:(pair + 1) * 128].rearrange("p (h d) -> p h d", h=2),
                )
                if pair == 0:
                    nc.vector.memset(vt[j][:, :, 64:65], 1.0)

            # ---------------- drip scheduler ----------------
            # PE-cost estimates (ns) for slot packing
            C_QKQ, C_V, C_OP, C_BC, C_ATT = 440, 560, 440, 220, 432

            quanta = []   # dicts: ready, dl, cost, fn, lane, vj

            def q_add(ready, dl, cost, fn, lane=None, vj=None):
                quanta.append(dict(ready=ready, dl=dl, cost=cost, fn=fn,
                                   lane=lane, vj=vj, qk=None, emitted=False))

            # data arrival estimates in global-slot units (slot 0 = first exp)
            R_SC = [-4, 3, 5, 8]
            R_WV = 1

            qk_done = {}

            def qk_thunks(dst, w, b, scn, fc, ready, dl):
                # four quarters; single global "qk" lane enforces strict
                # ordering (qkpool has one buffer — interleaving deadlocks)
                for qq in range(4):
                    q_add(ready, dl - (3 - qq), C_QKQ,
                          lambda qq=qq: qk_quarter(dst, w, b, scn, fc, qq),
                          lane="qk")
                    quanta[-1]["qk"] = (id(dst), scn)

            K0 = (kt[0], wkt, bk_sb, 0)
            Q0 = (qt[0], wqt, bq_sb, 0)
            K1 = (kt[1], wkt, bk_sb, 1)
            Q1 = (qt[1], wqt, bq_sb, 1)

            # insertion order = qk-lane order; must respect both deadlines
            # and (approximately) data arrival
            for (dst, w, b_, fc), scn, dl in (
                (K0, 2, 8), (K0, 3, 12),
                (Q0, 2, 31), (Q0, 3, 47),
                (K1, 0, 62), (Q1, 0, 63),
                (K1, 1, 68), (K1, 2, 72), (K1, 3, 76),
                (Q1, 1, 79), (Q1, 2, 95), (Q1, 3, 111),
            ):
                rd = R_SC[scn] + (20 if dst in (kt[1], qt[1]) else 0)
                qk_thunks(dst, w, b_, scn, fc, rd, dl)

            # v pair0: vt[j] wanted around block0 slot j (attnv defers)
            for j in range(N_JC):
                q_add(max(R_SC[j // 4], R_WV), j, C_V,
                      lambda j=j: v_proj(j, 0), vj=(0, j))
            # v pair1: vt[j] needed at block4 slot j
            for j in range(N_JC):
                q_add(max(R_SC[j // 4], 16), 40 + j, C_V,
                      lambda j=j: v_proj(j, 1), vj=(1, j))

            def out_proj_chunk(ic, ec, ss, tail=False, reserved=False, n=[0]):
                """One (128 s, 512 e) chunk of the partial output projection."""
                srow = ic * SC + ss * 128
                if tail or reserved:
                    # reuse the (now idle) score psum ring: same tag+shape
                    po = scpool.tile([128, 2 * SC], FP32, name="sc_ps")[:, 0:SC]
                else:
                    po = trpool.tile([128, SC], FP32, name="po", tag="tr")
                if not tail:
                    for fc in range(2):
                        nc.tensor.matmul(
                            po,
                            lhsT=at[fc][:, srow:srow + 128],
                            rhs=wo_sb[:, fc * D + ec * SC:fc * D + (ec + 1) * SC],
                            start=(fc == 0), stop=(fc == 1),
                        )
                else:
                    # heads C/D only (K=64 against the 64-row wo blocks 2/3);
                    # the at0 part was pre-staged during block 7
                    nc.tensor.matmul(
                        po, lhsT=at[1][0:64, srow:srow + 128],
                        rhs=wo_sb[0:64, 2 * D + ec * SC:2 * D + (ec + 1) * SC],
                        start=True, stop=False)
                    nc.tensor.matmul(
                        po, lhsT=stg_tail[0:64, ss * JC:(ss + 1) * JC],
                        rhs=wo_sb[0:64, 3 * D + ec * SC:3 * D + (ec + 1) * SC],
                        start=False, stop=True)
                stg = spool.tile([128, SC], BF16, name="ostg")
                n[0] += 1
                if tail:
                    nc.vector.tensor_add(stg, po, opa_stg[ec * 4 + ss])
                elif reserved:
                    nc.scalar.copy(stg, po)
                else:
                    nc.vector.tensor_copy(stg, po)
                nc.sync.dma_start(
                    out=out.ap()[srow:srow + 128, ec * SC:(ec + 1) * SC],
                    in_=stg,
                )

            # pre-staged at0 partials of the final ic's out_proj (dripped
            # during blocks 5-7; at[0] ic3 is normalized by block 4 slot 6)
            opa_stg = [apool.tile([128, SC], BF16, name=f"opa{i}")
                       for i in range(8)]

            def opa_chunk(idx):
                ec, ss = idx // 4, idx % 4
                srow = (N_SC - 1) * SC + ss * 128
                po = trpool.tile([128, SC], FP32, name="po", tag="tr")
                nc.tensor.matmul(
                    po, lhsT=at[0][:, srow:srow + 128],
                    rhs=wo_sb[:, ec * SC:(ec + 1) * SC])
                nc.vector.tensor_copy(opa_stg[idx], po)

            for idx in range(8):
                q_add(5 * 16 + 2, 6 * 16 + 8 + idx, 300,
                      lambda idx=idx: opa_chunk(idx))

            # out_proj drips: ic0 -> block5, ic1 -> block6, ic2 -> block7
            # (4 chunks dripped; 4 reserved for the tail), ic3 -> tail.
            for ic_prev, blk in ((0, 5), (1, 6)):
                for idx in range(8):
                    ec, ss = idx // 4, idx % 4
                    q_add(blk * 16 + 8, blk * 16 + 15, C_OP,
                          lambda ic=ic_prev, ec=ec, ss=ss: out_proj_chunk(ic, ec, ss))
            for ss in range(4):
                q_add(7 * 16 + 8, 7 * 16 + 15, C_OP,
                      lambda ss=ss: out_proj_chunk(2, 0, ss))

            # ---------------- normalize (deferred) ----------------
            # The softmax denominator Z sits in row 64 of the 96-partition
            # osb tiles. DVE 32x32 block-transposes spread it across 32
            # lanes for the (8-cycle-per-element) reciprocal and transpose
            # the result back to a single row — all SBUF-local, no DMA
            # bounce latency.
            norm_state = {}
            pending = [None]

            def n_t1(st):
                for h, osb in enumerate(st["osb"]):
                    nc.vector.transpose(zt_pair[h], osb[64:96, :])

            def n_recip(st):
                for h in range(2):
                    with nc.allow_low_precision(reason="softmax denom recip in bf16, matches prior bf16 cast"):
                        nc.vector.reciprocal(
                            rsp_pair[h].rearrange("p (c e) -> p c e", e=32)[:, :, 0],
                            zt_pair[h].rearrange("p (c e) -> p c e", e=32)[:, :, 0])

            def n_t2(st):
                for h in range(2):
                    nc.vector.transpose(rec_pair[h], rsp_pair[h])
                st["rec"] = rec_pair

            def n_mul_a(st):
                pair, ic = st["key"]
                i_sl = slice(ic * SC, (ic + 1) * SC)
                bc = trpool.tile([64, SC], FP32, name="bc", tag="tr")
                nc.tensor.matmul(bc, lhsT=ones0, rhs=st["rec"][0][0:1, :])
                nc.vector.tensor_mul(at[pair][0:64, i_sl], st["osb"][0][0:64, :], bc)

            def n_mul_b(st, tail=False):
                pair, ic = st["key"]
                i_sl = slice(ic * SC, (ic + 1) * SC)
                bc = trpool.tile([64, SC], FP32, name="bc", tag="tr")
                nc.tensor.matmul(bc, lhsT=ones0, rhs=st["rec"][1][0:1, :])
                if tail:
                    nc.vector.tensor_mul(stg_tail, st["osb"][1][0:64, :], bc)
                else:
                    stg = smpool.tile([64, SC], BF16, name="stg", tag="stg")
                    nc.vector.tensor_mul(stg, st["osb"][1][0:64, :], bc)
                    nc.sync.dma_start(out=at[pair][64:128, i_sl], in_=stg)

            # ---------------- main emission loop ----------------
            # prologue: k0/q0 s-chunk 0, halves interleaved so q0's dc0-3
            # matmuls run while the sc0 hi-half DMA is still landing. q0
            # accumulates in the transient pool (qkpool holds k0's group).
            k0_ps = qkpool.tile([128, SC], FP32, name="qk_ps", tag="qk")
            q0_ps = trpool.tile([128, SC], FP32, name="q0_ps", tag="tr")
            for dc in range(4):
                nc.tensor.matmul(k0_ps, lhsT=wkt[dc][:, 0:128],
                                 rhs=xt_sl(0, dc), start=(dc == 0), stop=False)
            for dc in range(4):
                nc.tensor.matmul(q0_ps, lhsT=wqt[dc][:, 0:128],
                                 rhs=xt_sl(0, dc), start=(dc == 0), stop=False)
            junk_mms(4)
            for dc in range(4, DCH):
                nc.tensor.matmul(k0_ps, lhsT=wkt[dc][:, 0:128],
                                 rhs=xt_sl(0, dc), start=False,
                                 stop=(dc == DCH - 1))
            nc.vector.tensor_scalar_add(kt[0][:, 0:SC], k0_ps, bk_sb[:, 0:1])

            for dc in range(4, DCH):
                nc.tensor.matmul(q0_ps, lhsT=wqt[dc][:, 0:128],
                                 rhs=xt_sl(0, dc), start=False,
                                 stop=(dc == DCH - 1))
            nc.vector.tensor_scalar_add(qt[0][:, 0:SC], q0_ps, bq_sb[:, 0:1])
            # s-chunk 1 of k0/q0 also fits the prologue: x sc1 lands before
            # the first exp, and pulling these out of block 0 removes the
            # slot-1..4 crunch and the forced q0sc1 burst at the block 0/1
            # boundary
            qk_full(kt[0], wkt, bk_sb, 1, 0)
            qk_full(qt[0], wqt, bq_sb, 1, 0)

            vt_ready = [[False] * N_JC, [False] * N_JC]
            # sc0 chunks of kt0/qt0 were emitted in the prologue
            qk_done[(id(kt[0]), 0)] = 4
            qk_done[(id(qt[0]), 0)] = 4
            qk_done[(id(kt[0]), 1)] = 4
            qk_done[(id(qt[0]), 1)] = 4

            def pop_ready(gslot, budget):
                got, blocked = [], set()
                cands = []
                for qd in quanta:
                    if qd["emitted"]:
                        continue
                    if qd["lane"] is not None:
                        if qd["lane"] in blocked:
                            continue
                        blocked.add(qd["lane"])
                    if qd["ready"] > gslot:
                        continue
                    cands.append(qd)
                cands.sort(key=lambda d: d["dl"])
                for qd in cands:
                    if budget <= 100:
                        break
                    if qd["cost"] > budget + 260:
                        continue
                    qd["emitted"] = True
                    budget -= qd["cost"]
                    got.append(qd)
                return got

            def emit_quantum(qd):
                qd["fn"]()
                if qd["vj"] is not None:
                    p, j = qd["vj"]
                    vt_ready[p][j] = True
                if qd["qk"] is not None:
                    qk_done[qd["qk"]] = qk_done.get(qd["qk"], 0) + 1

            def force_qk_until(dst, scn):
                # emit qk-lane entries in strict order until chunk complete
                while qk_done.get((id(dst), scn), 0) < 4:
                    for qd in quanta:
                        if not qd["emitted"] and qd["lane"] == "qk":
                            qd["emitted"] = True
                            emit_quantum(qd)
                            break
                    else:
                        raise RuntimeError("qk lane drained; chunk missing")

            def force_v(pair, j):
                for qd in quanta:
                    if not qd["emitted"] and qd["vj"] == (pair, j):
                        qd["emitted"] = True
                        emit_quantum(qd)
                        return

            BLOCKS = [(0, 0), (0, 1), (0, 2), (0, 3), (1, 0), (1, 1), (1, 2), (1, 3)]
            carry = [None]

            for b, (pair, ic) in enumerate(BLOCKS):
                i_sl = slice(ic * SC, (ic + 1) * SC)
                outA = aopool.tile([65, SC], FP32, name="outA", tag="ao")
                outB = aopool.tile([65, SC], FP32, name="outB", tag="ao")
                ex_tiles = {}
                attnv_done = [0]

                def attnv(jc, ex_tiles=ex_tiles, outA=outA, outB=outB, pair=pair):
                    nc.tensor.matmul(
                        outA, lhsT=vt[jc][:, 2 * pair, :],
                        rhs=ex_tiles[jc][:, 0:SC],
                        start=(jc == 0), stop=(jc == N_JC - 1),
                    )
                    nc.tensor.matmul(
                        outB, lhsT=vt[jc][:, 2 * pair + 1, :],
                        rhs=ex_tiles[jc][:, SC:2 * SC],
                        start=(jc == 0), stop=(jc == N_JC - 1),
                    )
                    attnv_done[0] += 1

                for jc in range(N_JC):
                    gslot = 16 * b + jc
                    force_qk_until(kt[pair], jc // 4)
                    force_qk_until(qt[pair], ic)
                    sc_ps = scpool.tile([128, 2 * SC], FP32, name="sc_ps")
                    nc.tensor.matmul(
                        sc_ps[:, 0:SC],
                        lhsT=kt[pair][0:64, jc * JC:(jc + 1) * JC],
                        rhs=qt[pair][0:64, i_sl],
                    )
                    nc.tensor.matmul(
                        sc_ps[:, SC:2 * SC],
                        lhsT=kt[pair][64:128, jc * JC:(jc + 1) * JC],
                        rhs=qt[pair][64:128, i_sl],
                    )
                    ex = epool.tile([128, 2 * SC], BF16, name="ex")
                    ex_tiles[jc] = ex
                    nc.scalar.activation(
                        ex, sc_ps, mybir.ActivationFunctionType.Exp,
                        bias=mb_sb[:, jc:jc + 1], scale=1.0 / np.sqrt(DH),
                    )
                    if carry[0] is not None:
                        # previous block's last attn@v + psum evacuation,
                        # emitted after this block's first scores so the
                        # boundary never stalls the exp stream
                        carry[0]()
                        carry[0] = None
                    # BISECT-A marker

                    budget = 1147 - 216
                    # previous block's normalize: pair-1 sts early (out_proj
                    # needs them next block); pair-0 sts late (clear of the
                    # early-block v/qk Vector congestion)
                    st = norm_state.get(b - 1)
                    if st is not None:
                        slots = ((8, 9, 10, 12, 13) if st["key"][0] == 0
                                 else (1, 2, 3, 5, 6))
                        if jc == slots[0]:
                            n_t1(st)
                        elif jc == slots[1]:
                            n_recip(st)
                        elif jc == slots[2]:
                            n_t2(st)
                        elif jc == slots[3]:
                            n_mul_a(st)
                            budget -= C_BC
                        elif jc == slots[4]:
                            n_mul_b(st)
                            budget -= C_BC

                    # drips first (they are data-ready; attn@v waits on the
                    # exp and would gate them in the in-order PE queue)
                    n_due = min(2, max(0, jc - attnv_done[0]))
                    budget -= C_ATT * n_due
                    for qd in pop_ready(gslot, max(budget, 0)):
                        emit_quantum(qd)
                    # attn@v catch-up: up to 2 per slot, lag >= 1 so the
                    # matmuls never wait on the just-issued exp
                    nv = 0
                    while (attnv_done[0] < jc and nv < 2
                           and vt_ready[pair][attnv_done[0]]):
                        attnv(attnv_done[0])
                        nv += 1

                # finish this block's attn@v except the last one or two
                # (force v projections if needed)
                while attnv_done[0] < N_JC - 1:
                    jj = attnv_done[0]
                    if not vt_ready[pair][jj]:
                        force_v(pair, jj)
                    attnv(jj)
                if not vt_ready[pair][N_JC - 1]:
                    force_v(pair, N_JC - 1)

                osbA = osb_ring[2 * b]
                osbB = osb_ring[2 * b + 1]

                def mk_carry(outA=outA, outB=outB, osbA=osbA, osbB=osbB,
                             last=(b == len(BLOCKS) - 1)):
                    def go():
                        if last:
                            # ScalarE is idle after the final exp
                            nc.scalar.copy(osbA[0:65, :], outA)
                            nc.scalar.copy(osbB[0:65, :], outB)
                        else:
                            nc.vector.tensor_copy(osbA[0:65, :], outA)
                            nc.vector.tensor_copy(osbB[0:65, :], outB)
                    return go

                attnv(N_JC - 1)   # BISECT-B: attnv15 inline, copies deferred
                carry[0] = mk_carry()
                st = {"key": (pair, ic), "osb": (osbA, osbB)}
                norm_state[b] = st

            # drain any un-emitted quanta (over-capacity leftovers)
            for qd in quanta:
                if not qd["emitted"]:
                    qd["emitted"] = True
                    emit_quantum(qd)

            # ---------------- tail ----------------
            st = norm_state[7]
            if carry[0] is not None:
                carry[0]()   # block 7's last attn@v + psum evacuation
                carry[0] = None
            # reserved out_proj chunks of ic2 + junk keep the PE warm while
            # the DVE runs the transpose-recip chain
            out_proj_chunk(2, 1, 0, reserved=True)
            n_t1(st)
            out_proj_chunk(2, 1, 1, reserved=True)
            junk_mms(4)
            n_recip(st)
            out_proj_chunk(2, 1, 2, reserved=True)
            n_t2(st)
            out_proj_chunk(2, 1, 3, reserved=True)
            junk_mms(8)
            n_mul_a(st)
            n_mul_b(st, tail=True)
            junk_mms(4)
            # final ic's output projection: heads C/D matmuls + the
            # pre-staged at0 partial added on DVE
            for ec in range(2):
                for ss in range(SC // 128):
                    out_proj_chunk(N_SC - 1, ec, ss, tail=True)

    nc.compile()
    return nc


_NC_CACHE = None


def _get_nc():
    global _NC_CACHE
    if _NC_CACHE is None:
        _NC_CACHE = build_kernel()
    return _NC_CACHE


def make_in_maps(inputs):
    x = np.asarray(inputs["x"], dtype=np.float32)
    mask = np.asarray(inputs["mask"])
    Wq = np.asarray(inputs["Wq"], dtype=np.float32)
    bq = np.asarray(inputs["bq"], dtype=np.float32)
    Wk = np.asarray(inputs["Wk"], dtype=np.float32)
    bk = np.asarray(inputs["bk"], dtype=np.float32)
    Wv = np.asarray(inputs["Wv"], dtype=np.float32)
    bv = np.asarray(inputs["bv"], dtype=np.float32)
    Wo = np.asarray(inputs["Wo"], dtype=np.float32)

    bf = ml_dtypes.bfloat16

    def pack_dxf(wT):  # (1024, FL) -> (128, 8*FL): d-chunks side by side
        return np.ascontiguousarray(
            wT.reshape(DCH, 128, FL).transpose(1, 0, 2).reshape(128, DCH * FL)
        )

    def pack_wo4(woT):  # (256, D) -> (128, 4*D)
        blk = np.zeros((128, 4 * D), dtype=woT.dtype)
        blk[:, 0:D] = woT[0:128]
        blk[:, D:2 * D] = woT[128:256]
        blk[0:64, 2 * D:3 * D] = woT[128:192]
        blk[0:64, 3 * D:4 * D] = woT[192:256]
        return blk

    in_maps = []
    for c in range(NCORES):
        b = c // GROUPS
        g = c % GROUPS
        fs, fe = g * FL, (g + 1) * FL
        xT = np.ascontiguousarray(x[b].T).astype(bf)  # (1024, 2048)
        xTp = np.ascontiguousarray(
            xT.reshape(DCH, 128, N_SC, SC).transpose(1, 2, 0, 3).reshape(128, -1)
        )
        bblob = np.zeros((128, BB_COLS), dtype=np.float32)
        bblob[:, BB_BK:BB_BK + 2] = bk[fs:fe].reshape(2, 128).T
        bblob[:, BB_BQ:BB_BQ + 2] = bq[fs:fe].reshape(2, 128).T
        bblob[:, BB_BV:BB_BV + FL] = np.tile(bv[fs:fe], (128, 1))
        bblob[:, BB_MB:BB_MB + N_JC] = (
            np.where(mask[b] == 0, np.float32(-1e9), np.float32(0.0))
            .astype(np.float32).reshape(N_JC, 128).T
        )
        wkq = np.concatenate([
            pack_dxf(Wk[fs:fe, :].T.astype(bf)),
            pack_dxf(Wq[fs:fe, :].T.astype(bf)),
        ], axis=1)
        in_maps.append({
            "xTp": xTp,
            "wkq": np.ascontiguousarray(wkq),
            "bb": bblob,
            "wv": pack_dxf(Wv[fs:fe, :].T.astype(bf)),
            "wo": pack_wo4(Wo[:, fs:fe].T.astype(bf)),
        })
    return in_maps


def kernel(x, mask, Wq, bq, Wk, bk, Wv, bv, Wo, bo):
    bo = np.asarray(bo, dtype=np.float32)
    nc = _get_nc()
    in_maps = make_in_maps(dict(x=x, mask=mask, Wq=Wq, bq=bq, Wk=Wk, bk=bk,
                                Wv=Wv, bv=bv, Wo=Wo, bo=bo))
    res = run_bass_kernel_spmd(nc, in_maps, core_ids=list(range(NCORES)))
    full = np.empty((B, S, D), dtype=np.float32)
    for b in range(B):
        acc = np.asarray(res.results[b * GROUPS]["out"], dtype=np.float32)
        for g in range(1, GROUPS):
            acc += np.asarray(res.results[b * GROUPS + g]["out"], dtype=np.float32)
        full[b] = acc + bo[None, :]
    return full
